# revision 1
# baseline (speedup 1.0000x reference)
"""Gated GQA attention block (B=2,S=2048,E=2048,H=16,HKV=2,D=256,RD=64) on 8 TRN2 cores.

Sharding: data-parallel on batch (2 groups of 4 cores); within a group,
tensor-parallel on query heads (4 heads/core). Each core computes its KV head's
k/v projection locally (duplicated across the 2 cores sharing a KV head).
o_proj is row-parallel; the all-reduce over the 4 cores of a group happens on
the host after gather.

Projection matmuls run bf16 x bf16 (hidden states and projection weights are
bf16: full PE rate, half the SBUF/HBM footprint); the attention chain
(k/q/v/exp/gated/o_proj operands) runs float32r (FP32 data truncated to ~FP22
in the PE array, full 1 cycle/row throughput at N>=256). PSUM accumulation is
fp32 throughout.
"""

import sys

if "/opt/trn_rl_repo" not in sys.path:
    sys.path.insert(0, "/opt/trn_rl_repo")

import ml_dtypes
import numpy as np

import concourse.bass as bass
import concourse.tile as tile
from concourse import bacc, mybir
from concourse.bass_utils import run_bass_kernel_spmd

F32 = mybir.dt.float32
F32R = mybir.dt.float32r
BF16 = mybir.dt.bfloat16
AF = mybir.ActivationFunctionType

S = 2048          # tokens per batch element
E = 2048          # model dim
D = 256           # head dim
RD = 64           # rope dims
NHC = 4           # q heads per core
HD = NHC * D      # per-core head dims (1024)
ECH = E // 128    # 16 contraction chunks
QCH = HD // 128   # 8 per-core q/g/o d-chunks
TT = 4            # 512-wide token tiles
NKC = S // 128    # 16 k chunks
NQC = S // 128    # 16 q chunks (oproj)


def _r(ap):
    return ap if ap.dtype in (F32R, BF16) else ap.bitcast(F32R)


def _body(tc, d):
    nc = tc.nc
    ts = bass.ts

    from contextlib import ExitStack

    stack = ExitStack()
    dram = stack.enter_context(tc.tile_pool(name="dram", bufs=1, space="DRAM"))
    qT_s = dram.tile([128, QCH, S], F32R, tag="qTs")
    gT_s = dram.tile([128, QCH, S], F32R, tag="gTs")
    gat_s = dram.tile([128, QCH, S], F32R, tag="gats")

    p_qg0 = stack.enter_context(tc.tile_pool(name="qg0", bufs=2))
    early_qg = {}

    p_kvres = stack.enter_context(tc.tile_pool(name="kvres", bufs=1))
    kt = p_kvres.tile([128, 2, S], F32R, tag="ktres")
    vt = p_kvres.tile([128, NKC, D], F32R, tag="vtres")
    mk = p_kvres.tile([128, 4, 512], F32, tag="mkres")
    nc.gpsimd.dma_start(mk[:], d["masks"].ap())

    p_ones = stack.enter_context(tc.tile_pool(name="ones", bufs=1))
    ones = p_ones.tile([128, 128], F32R, tag="ones")
    nc.scalar.dma_start(ones[:], d["ones"].ap())
    rotm = p_ones.tile([RD, RD], F32R, tag="rotm")
    nc.scalar.dma_start(rotm[:], d["rotm"].ap())

    psum = stack.enter_context(tc.tile_pool(name="psum", bufs=8, space="PSUM"))

    # ---------------- Phase 1: projections ----------------
    with (
        tc.tile_pool(name="xt", bufs=1) as p_xt,
        tc.tile_pool(name="w", bufs=2) as p_w,
        tc.tile_pool(name="wv", bufs=1) as p_wv,
        tc.tile_pool(name="osb", bufs=4) as p_osb,
        tc.tile_pool(name="trig", bufs=1) as p_trig,
        tc.tile_pool(name="rtmp", bufs=1) as p_rtmp,
    ):
        wv_t = p_wv.tile([128, ECH, D], BF16, tag="wv")
        for wh in range(4):
            nc.scalar.dma_start(
                wv_t[:, ts(wh, ECH // 4), :], d["wv"].ap()[:, ts(wh, ECH // 4), :]
            )

        cos_t = p_trig.tile([RD, S], F32, tag="cos")
        nc.scalar.dma_start(cos_t[:], d["cost"].ap())
        sin_t = p_trig.tile([RD, S], F32, tag="sin")
        nc.scalar.dma_start(sin_t[:], d["sint"].ap())

        xt = p_xt.tile([128, ECH, S], BF16, tag="xt")
        for ec in range(ECH):
            nc.sync.dma_start(xt[:, ec, :], d["xt"].ap()[:, ec, :])

        def rope(ot, t):
            # ot[0:64, :512] holds rope dims (partition = d).  rot = R @ x via
            # PE (keeps every DVE op partition-aligned), then
            # ot[0:64] = x*cos + rot*sin.
            rp = psum.tile([RD, 512], F32, tag="ps")
            nc.tensor.matmul(
                rp[:], _r(rotm[:]), _r(ot[0:RD, :]), start=True, stop=True
            )
            tmp = p_rtmp.tile([RD, 512], F32, tag="rt")
            nc.vector.tensor_mul(tmp[:], ot[0:RD, :], cos_t[:, ts(t, 512)])
            nc.vector.tensor_mul(ot[0:RD, :], rp[:], sin_t[:, ts(t, 512)])
            nc.vector.tensor_add(ot[0:RD, :], ot[0:RD, :], tmp[:])

        def proj_chunk(w_ap, dst, idx, kind):
            wt = p_w.tile([128, ECH, 128], BF16, tag="w")
            nc.scalar.dma_start(wt[:], w_ap)
            pss = []
            for t in range(TT):
                pt = psum.tile([128, 512], F32, tag="ps")
                pss.append(pt)
            for ec in range(ECH):
                for t in range(TT):
                    nc.tensor.matmul(
                        pss[t][:],
                        _r(wt[:, ec, :]),
                        _r(xt[:, ec, ts(t, 512)]),
                        start=(ec == 0),
                        stop=(ec == ECH - 1),
                    )
            for t in range(TT):
                if kind == "k":
                    kslice = kt[:, idx, ts(t, 512)]
                    nc.scalar.copy(kslice[:], pss[t][:])
                    if idx == 0:
                        rope(kslice, t)
                    continue
                ot = p_osb.tile([128, 512], F32R, tag="osb")
                if kind == "g":
                    nc.scalar.activation(ot[:], pss[t][:], AF.Sigmoid)
                else:
                    nc.scalar.copy(ot[:], pss[t][:])
                if kind == "q" and idx % 2 == 0:
                    rope(ot, t)
                nc.sync.dma_start(dst[:, idx, ts(t, 512)], ot[:])

        # v first, ec-outer in two 8-bank PSUM waves: its LDW-bound matmuls
        # consume each xt chunk as it streams in, filling the DMA ramp.
        for wave in range(2):
            pss = []
            for i in range(8):
                pv = psum.tile([128, D], F32, tag="ps")
                pss.append(pv)
            for ec in range(ECH):
                for i in range(8):
                    tcn = wave * 8 + i
                    nc.tensor.matmul(
                        pss[i][:],
                        _r(xt[:, ec, ts(tcn, 128)]),
                        _r(wv_t[:, ec, :]),
                        start=(ec == 0),
                        stop=(ec == ECH - 1),
                    )
            for i in range(8):
                tcn = wave * 8 + i
                nc.scalar.copy(vt[:, tcn, :], pss[i][:])

        for j in range(2):
            proj_chunk(d["wk"].ap()[j], None, j, "k")

        for h in range(NHC):
            for jj in (2 * h, 2 * h + 1):
                proj_chunk(d["wq"].ap()[jj], qT_s, jj, "q")
            for jj in (2 * h, 2 * h + 1):
                proj_chunk(d["wg"].ap()[jj], gT_s, jj, "g")
            if h < 2:
                # prefetch this head's (qq=TT-1) attention operands now: the
                # transfer overlaps the remaining projection work instead of
                # stalling the PE at the phase boundary.
                eq = p_qg0.tile([128, 2, 512], F32R, tag="qt0")
                nc.sync.dma_start(
                    eq[:], qT_s[:, 2 * h : 2 * h + 2, ts(TT - 1, 512)]
                )
                eg = p_qg0.tile([128, 2, 512], F32R, tag="gt0")
                nc.sync.dma_start(
                    eg[:], gT_s[:, 2 * h : 2 * h + 2, ts(TT - 1, 512)]
                )
                early_qg[h] = (eq, eg)

    # ---------------- Phase 2: attention ----------------
    with (
        tc.tile_pool(name="wo", bufs=1) as p_wo,
        tc.tile_pool(name="qg", bufs=4) as p_qg,
        tc.tile_pool(name="exp", bufs=6) as p_exp,
        tc.tile_pool(name="gat", bufs=4) as p_gat,
    ):
        wo_t = p_wo.tile([128, QCH, E], F32R, tag="wo")
        nc.gpsimd.dma_start(wo_t[:], d["wo"].ap())

        with (
            tc.tile_pool(name="gd", bufs=3) as p_gd,
            tc.tile_pool(name="ob", bufs=4) as p_ob,
        ):

            def oproj_block(qqd):
                for qc in range(4 * qqd, 4 * qqd + 4):
                    gd = p_gd.tile([128, QCH, 128], F32R, tag="gd")
                    nc.sync.dma_start(gd[:], gat_s[:, :, ts(qc, 128)])
                    for et in range(4):
                        op = psum.tile([128, 512], F32, tag="ps")
                        for hc in range(QCH):
                            nc.tensor.matmul(
                                op[:],
                                _r(gd[:, hc, :]),
                                _r(wo_t[:, hc, ts(et, 512)]),
                                start=(hc == 0),
                                stop=(hc == QCH - 1),
                            )
                        ob = p_ob.tile([128, 512], F32, tag="ob")
                        nc.scalar.copy(ob[:], op[:])
                        nc.sync.dma_start(d["out"].ap()[qc][:, ts(et, 512)], ob[:])

            qq_order = list(range(TT - 1, -1, -1))  # dense columns first
            for oi, qq in enumerate(qq_order):
                for h in range(NHC):
                    if qq == TT - 1 and h in early_qg:
                        qt, gt = early_qg[h]
                    else:
                        qt = p_qg.tile([128, 2, 512], F32R, tag="qt")
                        nc.sync.dma_start(
                            qt[:], qT_s[:, 2 * h : 2 * h + 2, ts(qq, 512)]
                        )
                        gt = p_qg.tile([128, 2, 512], F32R, tag="gt")
                        nc.sync.dma_start(
                            gt[:], gT_s[:, 2 * h : 2 * h + 2, ts(qq, 512)]
                        )
                    nk = 4 * qq + 4
                    av0 = psum.tile([128, 512], F32, tag="ps")
                    av1 = psum.tile([128, 512], F32, tag="ps")
                    sm = psum.tile([128, 512], F32, tag="ps")
                    for kk in range(nk):
                        sp = psum.tile([128, 512], F32, tag="ps")
                        nc.tensor.matmul(
                            sp[:], _r(kt[:, 0, ts(kk, 128)]), _r(qt[:, 0, :]),
                            start=True, stop=False,
                        )
                        nc.tensor.matmul(
                            sp[:], _r(kt[:, 1, ts(kk, 128)]), _r(qt[:, 1, :]),
                            start=False, stop=True,
                        )
                        ex = p_exp.tile([128, 512], F32R, tag="ex")
                        nc.scalar.activation(ex[:], sp[:], AF.Exp, scale=0.0625)
                        j = kk - (nk - 4)
                        if j >= 0:
                            nc.vector.tensor_mul(ex[:], ex[:], mk[:, j, :])
                        st, en = (kk == 0), (kk == nk - 1)
                        nc.tensor.matmul(
                            av0[:], _r(vt[:, kk, 0:128]), _r(ex[:]), start=st, stop=en
                        )
                        nc.tensor.matmul(
                            av1[:], _r(vt[:, kk, 128:256]), _r(ex[:]), start=st, stop=en
                        )
                        nc.tensor.matmul(
                            sm[:], _r(ones[:]), _r(ex[:]), start=st, stop=en
                        )
                    rec = p_gat.tile([128, 512], F32, tag="rec")
                    nc.vector.reciprocal(rec[:], sm[:])
                    for c, avc in enumerate((av0, av1)):
                        g1 = p_gat.tile([128, 512], F32R, tag="g1")
                        nc.vector.tensor_mul(g1[:], avc[:], gt[:, c, :])
                        nc.vector.tensor_mul(g1[:], g1[:], rec[:])
                        nc.sync.dma_start(gat_s[:, 2 * h + c, ts(qq, 512)], g1[:])

                # o_proj pipelined one column behind attention: by the time
                # the PE reaches these matmuls, the gat_s round trip for the
                # previous column has completed, so no PE stall.
                if oi > 0:
                    oproj_block(qq_order[oi - 1])
            oproj_block(qq_order[-1])

    stack.close()


def build_nc():
    nc = bacc.Bacc("TRN2", target_bir_lowering=False, debug=False)
    d = {}
    d["xt"] = nc.dram_tensor("xt", [128, ECH, S], BF16, kind="ExternalInput")
    d["wq"] = nc.dram_tensor("wq", [QCH, 128, ECH, 128], BF16, kind="ExternalInput")
    d["wg"] = nc.dram_tensor("wg", [QCH, 128, ECH, 128], BF16, kind="ExternalInput")
    d["wk"] = nc.dram_tensor("wk", [2, 128, ECH, 128], BF16, kind="ExternalInput")
    d["wv"] = nc.dram_tensor("wv", [128, ECH, D], BF16, kind="ExternalInput")
    d["wo"] = nc.dram_tensor("wo", [128, QCH, E], F32R, kind="ExternalInput")
    d["cost"] = nc.dram_tensor("cost", [RD, S], F32, kind="ExternalInput")
    d["sint"] = nc.dram_tensor("sint", [RD, S], F32, kind="ExternalInput")
    d["masks"] = nc.dram_tensor("masks", [128, 4, 512], F32, kind="ExternalInput")
    d["rotm"] = nc.dram_tensor("rotm", [RD, RD], F32R, kind="ExternalInput")
    d["ones"] = nc.dram_tensor("ones", [128, 128], F32R, kind="ExternalInput")
    d["out"] = nc.dram_tensor("out", [NQC, 128, E], F32, kind="ExternalOutput")
    with tile.TileContext(nc) as tc:
        _body(tc, d)
    nc.compile()
    return nc


_NC_CACHE = None


def _get_nc():
    global _NC_CACHE
    if _NC_CACHE is None:
        _NC_CACHE = build_nc()
    return _NC_CACHE


def _rope_tables():
    inv = 1.0 / (10000.0 ** (np.arange(0, RD, 2, dtype=np.float32) / np.float32(RD)))
    t = np.arange(S, dtype=np.float32)
    freqs = np.outer(t, inv).astype(np.float32)          # [S, RD/2]
    emb = np.concatenate([freqs, freqs], axis=1)         # [S, RD]
    return (
        np.ascontiguousarray(np.cos(emb).astype(np.float32).T),
        np.ascontiguousarray(np.sin(emb).astype(np.float32).T),
    )


def _rotm():
    r = np.zeros((RD, RD), dtype=np.float32)  # r[j, d] = R[d, j], rot = R @ x
    half = RD // 2
    for dd in range(half):
        r[dd + half, dd] = -1.0
    for dd in range(half, RD):
        r[dd - half, dd] = 1.0
    return r


def _masks():
    p = np.arange(128)[:, None, None]
    j = np.arange(4)[None, :, None]
    s = np.arange(512)[None, None, :]
    return ((p + 128 * j) <= s).astype(np.float32)


def _prep_in_maps(hidden_states, Wq, Wk, Wv, Wg, Wo):
    cosT, sinT = _rope_tables()
    masks = _masks()
    maps = []
    for c in range(8):
        b, t = c // 4, c % 4
        hq0, kvh = 4 * t, (t // 2)
        cols = slice(hq0 * D, (hq0 + NHC) * D)
        kcols = slice(kvh * D, (kvh + 1) * D)
        x = hidden_states[b]  # [S, E]
        m = {
            "xt": np.ascontiguousarray(
                x.T.reshape(ECH, 128, S).transpose(1, 0, 2)
            ).astype(ml_dtypes.bfloat16),
            "wq": np.ascontiguousarray(
                Wq[:, cols].reshape(ECH, 128, QCH, 128).transpose(2, 1, 0, 3)
            ).astype(ml_dtypes.bfloat16),
            "wg": np.ascontiguousarray(
                Wg[:, cols].reshape(ECH, 128, QCH, 128).transpose(2, 1, 0, 3)
            ).astype(ml_dtypes.bfloat16),
            "wk": np.ascontiguousarray(
                Wk[:, kcols].reshape(ECH, 128, 2, 128).transpose(2, 1, 0, 3)
            ).astype(ml_dtypes.bfloat16),
            "wv": np.ascontiguousarray(
                Wv[:, kcols].reshape(ECH, 128, D).transpose(1, 0, 2)
            ).astype(ml_dtypes.bfloat16),
            "wo": np.ascontiguousarray(
                Wo[cols, :].reshape(QCH, 128, E).transpose(1, 0, 2)
            ),
            "cost": cosT,
            "sint": sinT,
            "masks": masks,
            "rotm": _rotm(),
            "ones": np.ones((128, 128), dtype=np.float32),
        }
        maps.append(m)
    return maps


def _run(inputs, trace=False, trace_cores=None, tmpdir=None):
    nc = _get_nc()
    in_maps = _prep_in_maps(**inputs)
    kw = {}
    if trace:
        kw = dict(trace=True, trace_cores=trace_cores, tmpdir=tmpdir)
    res = run_bass_kernel_spmd(nc, in_maps, list(range(8)), **kw)
    outs = [res.results[c]["out"].reshape(S, E) for c in range(8)]
    full = np.stack(
        [
            outs[0] + outs[1] + outs[2] + outs[3],
            outs[4] + outs[5] + outs[6] + outs[7],
        ]
    ).astype(np.float32)
    return full, res


def kernel(hidden_states, Wq, Wk, Wv, Wg, Wo):
    full, _ = _run(
        dict(hidden_states=np.asarray(hidden_states, dtype=np.float32),
             Wq=np.asarray(Wq, dtype=np.float32),
             Wk=np.asarray(Wk, dtype=np.float32),
             Wv=np.asarray(Wv, dtype=np.float32),
             Wg=np.asarray(Wg, dtype=np.float32),
             Wo=np.asarray(Wo, dtype=np.float32))
    )
    return full

